# revision 4
# baseline (speedup 1.0000x reference)
"""BiDAF attention-flow kernel for Trainium2 (Bass/Tile), SPMD over 8 cores.

Math (per batch element b, one NeuronCore each):
    s[c,j]  = sc[c] + sq[j] + cq[c,j]            (scalar biases cancel)
    a       = softmax_j(s)
    c2q     = a @ e1                              (C,H)
    b_att   = softmax_c(max_j s)
    q2c     = b_att @ e2                          (H,)
    out     = [e2, c2q, e2*c2q, e2*q2c] @ w_red.T + b_red

Key tricks vs the obvious mapping:
  * sc[c] = sum_h wc[h]*e2t[h,c] is folded into the phase-A weights:
    e1w[h,j] = wcq[h]*e1t[h,j] + wc[h], so s (incl. sc) comes out of the
    same matmuls and max_j exp(s) IS the b_att numerator E — no separate
    exp(sc) pass, no broadcast multiplies.
  * c2q is kept UNNORMALIZED (U2 = P @ e1); the 1/L row softmax scale is
    applied once per output c-tile as a per-partition activation scale at
    the blocks-2/3 PSUM eviction (block2 = c2q @ w2, block3 = (e2*c2q) @ w3
    share the same 1/L[c] factor). Phase-B evictions become plain Scalar
    copies, freeing the Vector engine.
  * 1/L needs c on partitions for that: the L row (ones-matmul over P_T)
    roundtrips through a DRAM scratch with a transposing rearrange.
  * reduction layer runs as two passes with no duplicated work:
    pass1 = blocks 2+3 (needs only U2/m3), pass2 = e2-block with
    wsum = wrt[0:6] + q2c-folded wrt[18:24] (+bias), summed on eviction.
    q2c (gpsimd max-allreduce + mul-reduce accumulation) is computed
    during phase B / pass1, far off the critical path.
  * Phase B is ch-outer so U2 chunks complete early, letting m3 = e2*c2q
    DVE muls and pass1 start without a bubble.
  * DMAs stream in phase-A consumption order (wpk, e1t, e2t per-ht, e1,
    wrt); phase A starts as soon as the first e2t stripe lands.

Phases (each fully unrolled; Tile pipelines across them):
    warm + sq -> A (s matmuls, exp, running max) -> L -> B (ch-outer)
    -> pass1 (blocks 2+3, 1/L on evict) -> pass2 (e2 block + bias, add, store)

Host does sharding/layout only: batch split, transposes, bf16 casts.
"""

import numpy as np
import ml_dtypes

B, Q, C, H, OUT = 8, 512, 2048, 768, 300
HT, JT, CT = H // 128, Q // 128, C // 128  # 6, 4, 16
NCH, CHW = 4, 512  # c chunks
CPT = 4  # c-tiles per chunk
NWARM = 10

bf16 = ml_dtypes.bfloat16

_CACHE = {}


def _build_bass():
    import concourse.tile as tile
    from concourse import mybir, bass_isa, library_config, bacc

    f32 = mybir.dt.float32
    b16 = mybir.dt.bfloat16
    AF = mybir.ActivationFunctionType
    OP = mybir.AluOpType

    nc = bacc.Bacc("TRN2", target_bir_lowering=False, debug=False)

    e1_d = nc.dram_tensor("e1", [Q, H], b16, kind="ExternalInput").ap()
    e1t_d = nc.dram_tensor("e1t", [H, Q], b16, kind="ExternalInput").ap()
    e2t_d = nc.dram_tensor("e2t", [H, C], b16, kind="ExternalInput").ap()
    wrt_d = nc.dram_tensor("wrt", [4 * H, OUT], b16, kind="ExternalInput").ap()
    wpk_d = nc.dram_tensor("wpk", [128, 3 * HT], f32, kind="ExternalInput").ap()
    bred_d = nc.dram_tensor("bred", [1, OUT], b16, kind="ExternalInput").ap()
    lscr_d = nc.dram_tensor("lscr", [1, C], f32, kind="Internal").ap()
    out_d = nc.dram_tensor("out", [C, OUT], f32, kind="ExternalOutput").ap()

    with tile.TileContext(nc) as tc:
        with (
            tc.tile_pool(name="singles", bufs=1) as singles,
            tc.tile_pool(name="amo", bufs=4) as amop,
            tc.tile_pool(name="odma", bufs=4) as odp,
            tc.tile_pool(name="ps_mm", bufs=6, space="PSUM") as ps_mm,
            tc.tile_pool(name="ps_out", bufs=2, space="PSUM") as ps_out,
        ):
            # gpsimd: need the 'attn' ucode library for partition_all_reduce
            nc.gpsimd.load_library(library_config.attn)

            # ---- persistent SBUF tensors -------------------------------
            e1_sb = singles.tile([128, JT, H], b16)      # emb1, j on parts
            e1t_sb = singles.tile([128, HT, Q], b16)     # emb1.T, h on parts
            e1w_sb = singles.tile([128, HT, Q], b16)     # wcq*e1T + wc
            e2t_sb = singles.tile([128, HT, C], b16)     # emb2.T, h on parts
            wrt_sb = singles.tile([128, 24, OUT], b16)   # w_red.T, k on parts
            wq4_sb = singles.tile([128, HT, OUT], b16)   # q2c-folded wrT tail
            wsum_sb = singles.tile([128, HT, OUT], b16)  # wrT[0:6] + wq4T
            wpk_sb = singles.tile([128, 3 * HT], f32)
            wq_sb = singles.tile([128, HT], b16)
            bred_sb = singles.tile([1, OUT], b16)
            ones_mat = singles.tile([128, 128], b16)
            ones_row_b = singles.tile([1, 128], b16)
            ones_row_f = singles.tile([1, 128], f32)
            sq_sb = singles.tile([128, JT], f32)         # sq as columns
            pt_sb = singles.tile([128, JT, NCH, CHW], b16)  # P_T = exp(s+sq)
            u2_sb = singles.tile([128, HT, C], b16)      # c2qT, UNnormalized
            m3_sb = singles.tile([128, HT, C], b16)      # e2t * u2
            macc = singles.tile([128, C], b16)           # col-max of P_T
            mall = singles.tile([128, C], b16)           # = E after all-reduce
            lrow_sb = singles.tile([1, C], f32)          # L as a row
            ltc_sb = singles.tile([128, CT], f32)        # L cols (c on parts)
            rcT_sb = singles.tile([128, CT], f32)        # 1/L cols
            s_parts = singles.tile([1, NCH], f32)
            s_sum = singles.tile([1, 1], f32)
            rs_sum = singles.tile([1, 1], f32)
            rs_col = singles.tile([128, 1], f32)
            u_sb = singles.tile([128, HT, NCH], f32)     # unnormalized q2c
            q2c_sb = singles.tile([128, HT], f32)
            out_sb = singles.tile([128, CT, OUT], f32)   # pass-1 partials

            # ---- loads in phase-A consumption order --------------------
            nc.sync.dma_start(out=wpk_sb, in_=wpk_d)
            e1t_r = e1t_d.rearrange("(t p) j -> p t j", p=128)
            e2t_r = e2t_d.rearrange("(t p) c -> p t c", p=128)
            for ht in range(HT):
                nc.sync.dma_start(out=e1t_sb[:, ht, :], in_=e1t_r[:, ht, :])
                nc.sync.dma_start(out=e2t_sb[:, ht, :], in_=e2t_r[:, ht, :])
            nc.sync.dma_start(
                out=e1_sb, in_=e1_d.rearrange("(t p) h -> p t h", p=128)
            )
            nc.sync.dma_start(
                out=wrt_sb, in_=wrt_d.rearrange("(t p) o -> p t o", p=128)
            )
            nc.sync.dma_start(out=bred_sb, in_=bred_d)

            wcq_sb = wpk_sb[:, 0:HT]
            nc.vector.memset(ones_mat, 1.0)
            nc.vector.memset(ones_row_b, 1.0)
            nc.vector.memset(ones_row_f, 1.0)
            nc.vector.memset(macc, 0.0)
            nc.vector.tensor_copy(wq_sb, wpk_sb[:, 2 * HT : 3 * HT])
            # e1w = wcq (per h) * e1T + wc  (sc rides along in the matmuls)
            for ht in range(HT):
                nc.vector.tensor_scalar(
                    e1w_sb[:, ht, :], e1t_sb[:, ht, :],
                    wcq_sb[:, ht : ht + 1],
                    wpk_sb[:, HT + ht : HT + ht + 1],
                    OP.mult, OP.add,
                )

            # HAM warm-up: ramp the PE clock while the first inputs stream
            wps = ps_mm.tile([128, CHW], f32, tag="mm", name="warm")
            for _ in range(NWARM):
                nc.tensor.matmul(wps[:, 0:128], ones_mat, ones_mat,
                                 start=True, stop=True)
            nc.vector.tensor_copy(rs_col, wps[:, 0:1])

            # ---- sq columns (tiny, feeds exp bias) ---------------------
            for jt in range(JT):
                ps = ps_mm.tile([128, CHW], f32, tag="mm")
                for ht in range(HT):
                    nc.tensor.matmul(
                        ps[:, 0:1],
                        e1t_sb[:, ht, jt * 128 : (jt + 1) * 128],
                        wq_sb[:, ht : ht + 1],
                        start=(ht == 0),
                        stop=(ht == HT - 1),
                    )
                nc.vector.tensor_copy(sq_sb[:, jt : jt + 1], ps[:, 0:1])

            # ---- phase A: sT matmuls, exp, running max -----------------
            # jt outer / ht mid / ch inner: e1w stationary tile is reused
            # across the 4 chunks (one LDWEIGHTS per (jt, ht)); ht-major
            # consumption matches the e2t DMA stream order.
            for jt in range(JT):
                sps = [
                    ps_mm.tile([128, CHW], f32, tag="mm", name=f"sps{jt}_{i}")
                    for i in range(NCH)
                ]
                for ht in range(HT):
                    for ch in range(NCH):
                        nc.tensor.matmul(
                            sps[ch],
                            e1w_sb[:, ht, jt * 128 : (jt + 1) * 128],
                            e2t_sb[:, ht, ch * CHW : (ch + 1) * CHW],
                            start=(ht == 0),
                            stop=(ht == HT - 1),
                        )
                for ch in range(NCH):
                    csl = slice(ch * CHW, (ch + 1) * CHW)
                    nc.scalar.activation(
                        out=pt_sb[:, jt, ch, :], in_=sps[ch], func=AF.Exp,
                        bias=sq_sb[:, jt : jt + 1], scale=1.0,
                    )
                    nc.vector.tensor_max(
                        macc[:, csl], macc[:, csl], pt_sb[:, jt, ch, :]
                    )

            # ---- b_att numerator: E = all-reduce-max(macc) -------------
            # (sc is inside s, so max_j exp(s) IS the numerator)
            nc.gpsimd.partition_all_reduce(
                mall, macc, channels=128, reduce_op=bass_isa.ReduceOp.max
            )

            # ---- L row via ones-matmul, transpose to c-on-partitions ---
            for ch in range(NCH):
                csl = slice(ch * CHW, (ch + 1) * CHW)
                lps = ps_mm.tile([128, CHW], f32, tag="mm", name=f"lps{ch}")
                for jt in range(JT):
                    nc.tensor.matmul(
                        lps, ones_mat, pt_sb[:, jt, ch, :],
                        start=(jt == 0), stop=(jt == JT - 1),
                    )
                nc.vector.tensor_copy(lrow_sb[0:1, csl], lps[0:1, :])
            nc.sync.dma_start(out=lscr_d, in_=lrow_sb[0:1, :])
            nc.sync.dma_start(
                out=ltc_sb, in_=lscr_d.rearrange("o (t p) -> p (o t)", p=128)
            )
            nc.vector.reciprocal_approx_fast(out=rcT_sb, in_=ltc_sb)

            # ---- phase B: c2qT (unnormalized) matmuls, ch-outer --------
            # ch outer so each U2 chunk completes early; evictions are
            # plain copies on the Scalar engine (1/L applied later).
            for ch in range(NCH):
                csl = slice(ch * CHW, (ch + 1) * CHW)
                for ht in range(HT):
                    cps = ps_mm.tile([128, CHW], f32, tag="mm",
                                     name=f"cps{ch}_{ht}")
                    for jt in range(JT):
                        nc.tensor.matmul(
                            cps,
                            e1_sb[:, jt, ht * 128 : (ht + 1) * 128],
                            pt_sb[:, jt, ch, :],
                            start=(jt == 0), stop=(jt == JT - 1),
                        )
                    nc.scalar.copy(u2_sb[:, ht, csl], cps)

            # ---- q2c accumulation on DVE, interleaved ------------------
            # amr: u[h, ch] = sum_c e2t[h,c]*E[c]; m3 = e2t*U2 as soon as
            # each U2 chunk lands. Emission order = DVE execution order.
            amr_jobs = [
                (ht, ch2) for ch2 in range(NCH) for ht in range(HT)
            ]

            def emit_amr(n):
                for _ in range(n):
                    if not amr_jobs:
                        return
                    ht, ch2 = amr_jobs.pop(0)
                    csl2 = slice(ch2 * CHW, (ch2 + 1) * CHW)
                    amo = amop.tile(
                        [128, CHW], b16, tag="amo", name=f"am{ch2}_{ht}"
                    )
                    nc.vector.affine_mul_reduce(
                        out=amo,
                        accum_out=u_sb[:, ht, ch2 : ch2 + 1],
                        in0=e2t_sb[:, ht, csl2],
                        in1=mall[:, csl2],
                        scale=1.0,
                        bias=0.0,
                    )

            def emit_m3(ch):
                csl = slice(ch * CHW, (ch + 1) * CHW)
                for ht in range(HT):
                    nc.vector.tensor_mul(
                        m3_sb[:, ht, csl], e2t_sb[:, ht, csl],
                        u2_sb[:, ht, csl],
                    )

            # S = sum_c E (for q2c normalization), as soon as mall lands
            for ch in range(NCH):
                csl = slice(ch * CHW, (ch + 1) * CHW)
                nc.vector.reduce_sum(
                    out=s_parts[:, ch : ch + 1], in_=mall[0:1, csl],
                    axis=mybir.AxisListType.X,
                )
            emit_amr(6)
            emit_m3(0)
            emit_amr(6)
            emit_m3(1)
            emit_amr(6)
            emit_m3(2)
            emit_amr(6)
            emit_m3(3)

            # ---- q2c finalize: q2c = U / S, fold into wrT tail ---------
            nc.vector.reduce_sum(
                out=s_sum, in_=s_parts, axis=mybir.AxisListType.X
            )
            nc.vector.reciprocal_approx_fast(out=rs_sum, in_=s_sum)
            rps = ps_out.tile([128, OUT], f32, tag="out")
            nc.tensor.matmul(
                rps[:, 0:1], ones_row_f, rs_sum, start=True, stop=True
            )
            nc.vector.tensor_copy(rs_col, rps[:, 0:1])
            nc.vector.reduce_sum(
                out=q2c_sb, in_=u_sb, axis=mybir.AxisListType.X
            )
            nc.vector.tensor_scalar_mul(q2c_sb, q2c_sb, rs_col)
            for ht in range(HT):
                nc.vector.tensor_scalar_mul(
                    wq4_sb[:, ht, :], wrt_sb[:, 18 + ht, :],
                    q2c_sb[:, ht : ht + 1],
                )
                nc.vector.tensor_add(
                    wsum_sb[:, ht, :], wq4_sb[:, ht, :], wrt_sb[:, ht, :]
                )

            # ---- pass 1: blocks 2+3 (c2q @ w2 + (e2*c2q) @ w3) ---------
            # both blocks share the 1/L[c] factor -> apply as per-partition
            # activation scale at PSUM eviction.
            for ct in range(CT):
                ch = ct // CPT
                tsl = slice(ct * 128, (ct + 1) * 128)
                ops = ps_out.tile([128, OUT], f32, tag="out", name=f"ops{ct}")
                for ht in range(HT):
                    nc.tensor.matmul(
                        ops, u2_sb[:, ht, tsl], wrt_sb[:, 6 + ht, :],
                        start=(ht == 0), stop=False,
                    )
                for ht in range(HT):
                    nc.tensor.matmul(
                        ops, m3_sb[:, ht, tsl], wrt_sb[:, 12 + ht, :],
                        start=False, stop=(ht == HT - 1),
                    )
                nc.scalar.activation(
                    out=out_sb[:, ct, :], in_=ops, func=AF.Copy,
                    bias=0.0, scale=rcT_sb[:, ct : ct + 1],
                )

            # ---- pass 2: e2 block with q2c-folded weights + bias -------
            for ct in range(CT):
                tsl = slice(ct * 128, (ct + 1) * 128)
                obs = ps_out.tile([128, OUT], f32, tag="out", name=f"obs{ct}")
                for ht in range(HT):
                    nc.tensor.matmul(
                        obs, e2t_sb[:, ht, tsl], wsum_sb[:, ht, :],
                        start=(ht == 0), stop=False,
                    )
                nc.tensor.matmul(
                    obs, ones_row_b, bred_sb, start=False, stop=True,
                )
                od = odp.tile([128, OUT], f32, tag="od", name=f"od{ct}")
                nc.vector.tensor_add(od, obs, out_sb[:, ct, :])
                nc.sync.dma_start(out=out_d[tsl, :], in_=od)

    nc.compile()
    return nc


def _get_nc():
    if "nc" not in _CACHE:
        _CACHE["nc"] = _build_bass()
    return _CACHE["nc"]


def _in_maps(emb1, emb2, w_c, b_c, w_q, b_q, w_cq, b_cq, w_red, b_red):
    # host-side sharding + layout only: batch split, transposes, bf16 casts
    emb1 = np.asarray(emb1, np.float32)
    emb2 = np.asarray(emb2, np.float32)
    wcq = np.asarray(w_cq, np.float32).reshape(HT, 128).T
    wc = np.asarray(w_c, np.float32).reshape(HT, 128).T
    wq = np.asarray(w_q, np.float32).reshape(HT, 128).T
    wpk = np.ascontiguousarray(np.concatenate([wcq, wc, wq], axis=1))
    wrt = np.ascontiguousarray(np.asarray(w_red, np.float32).T).astype(bf16)
    bred = np.asarray(b_red, np.float32).reshape(1, OUT).astype(bf16)
    maps = []
    for b in range(B):
        maps.append(
            {
                "e1": emb1[b].astype(bf16),
                "e1t": np.ascontiguousarray(emb1[b].T).astype(bf16),
                "e2t": np.ascontiguousarray(emb2[b].T).astype(bf16),
                "wrt": wrt,
                "wpk": wpk,
                "bred": bred,
            }
        )
    return maps


def run(inputs, trace=False):
    from concourse.bass_utils import run_bass_kernel_spmd

    nc = _get_nc()
    maps = _in_maps(**inputs)
    res = run_bass_kernel_spmd(nc, maps, list(range(B)), trace=trace)
    out = np.stack([res.results[b]["out"] for b in range(B)], axis=0)
    return out.astype(np.float32), res


def kernel(**inputs) -> np.ndarray:
    out, _ = run(inputs, trace=False)
    return out
